# revision 2
# baseline (speedup 1.0000x reference)
"""Trainium2 Bass kernel for the pairwise-classifier loss.

Math: per branch, logits = x @ W + b with only 2 classes, so everything
reduces to the logit difference d = x . (W[:,1]-W[:,0]) + (b[1]-b[0]).
With x a concat of two gathered feature vectors, d splits into a sum of
two per-position projections:
    lo[b,n] = sum_c feats[b,c,n] * w[c]      (w = first 256 rows of dW)
    hi[b,n] = sum_c feats[b,c,n] * w[256+c]  (last 256 rows)
Per pair r: d_pos = lo[pb,pi] + hi[pb,pj] + db
            d_negA = lo[nb,ni] + hi[pb,pi] + db
            d_negB = lo[nb,ni] + hi[pb,pj] + db
and the double-softmax CE row loss, with y = tanh((d+db)/2)
(= 2*sigmoid(d+db)-1), E = exp(y), L = ln(E+1):
    loss_neg = L           (softplus(y))
    loss_pos = L - y       (softplus(-y))
The permutation inputs are irrelevant: the loss is a mean over rows.

Device plan (8 cores, pairs data-parallel 12500/core, replicated
feature load -> no collectives):
  1. Each core streams all 16MB of features (f32->bf16 cast in DMA),
     8 per-b chunks double-buffered.
  2. PE per b: feats_block[128c,128n]^T @ Wp[128c,4] -> PSUM [128n,4m],
     m = (row_lo,row_hi,col_lo,col_hi); 2 k-tiles accumulated.  DVE
     de-interleaves row/col into two bf16 branch tables in SBUF, DMAed
     to DRAM as flat 64KB tables: row index (n%128)*128 + b*16 + n/128,
     d=2 (lo,hi).
  3. Per-branch table broadcast to all 128 partitions (8MB DMA each),
     col serialized after row so row gathers start early.
  4. Flat table offsets via DVE int ops; 12 ap_gather calls (6 idx
     streams x 2 halves) of 8-byte (lo,hi) bf16 rows; DVE adds form
     d tiles; ACT tanh/exp/ln with per-partition accum_out.
  5. Pad slots replicate table row 0; their uniform contribution is
     computed from a known pad slot and subtracted.  PE dot with the
     1/(3R*16) scale vector -> per-core partial; host sums 8 partials.
"""

import os

import numpy as np

import concourse.bass as bass
import concourse.bacc as bacc
import concourse.mybir as mybir
import concourse.tile as tile
from concourse.bass_utils import run_bass_kernel_spmd

F32 = mybir.dt.float32
BF16 = mybir.dt.bfloat16
I32 = mybir.dt.int32
I16 = mybir.dt.int16

B, C, N = 8, 256, 2048
R = 100000
NCORES = 8
PAIRS = R // NCORES          # 12500 pairs per core per branch
P = 128
GP = 128                     # gather tile partitions (one Q7 core per 16)
GK = 100                     # idx free width; 16*GK = 1600 pairs per Q7 core
NI = 16 * GK                 # ap_gather num_idxs per core
# per-Q7-core real pair counts (8 cores x 1600 slots = 12800 >= 12500)
N_REAL = [1563, 1563, 1563, 1563, 1562, 1562, 1562, 1562]
TROWS_BR = P * B * (N // P)  # 16384 (q, b, blk) rows per branch table
TELEMS_BR = TROWS_BR * 2     # bf16 elements per flat branch table


def _emit_weight_prep(nc, const, psmall, w_row, w_col, b_row, b_col):
    """Wp [128, 2kt*4m] bf16 projection weights and db128 [128, 2] f32
    holding (b[1]-b[0])/2 per branch broadcast to all partitions."""
    wr_raw = const.tile([P, 8], F32, tag="wr_raw")
    wc_raw = const.tile([P, 8], F32, tag="wc_raw")
    nc.sync.dma_start(
        out=wr_raw[:].rearrange("p (s t) -> p s t", s=4),
        in_=w_row[:].rearrange("(s p) t -> p s t", p=P),
    )
    nc.sync.dma_start(
        out=wc_raw[:].rearrange("p (s t) -> p s t", s=4),
        in_=w_col[:].rearrange("(s p) t -> p s t", p=P),
    )
    wdiff_r = const.tile([P, 4], F32, tag="wdiff_r")
    wdiff_c = const.tile([P, 4], F32, tag="wdiff_c")
    nc.vector.tensor_tensor(
        out=wdiff_r[:], in0=wr_raw[:, 1::2], in1=wr_raw[:, 0::2],
        op=mybir.AluOpType.subtract,
    )
    nc.vector.tensor_tensor(
        out=wdiff_c[:], in0=wc_raw[:, 1::2], in1=wc_raw[:, 0::2],
        op=mybir.AluOpType.subtract,
    )
    # Wp[:, kt*4 + m]: m = (row_lo, row_hi, col_lo, col_hi)
    wp = const.tile([P, 8], BF16, tag="wp")
    nc.vector.tensor_copy(out=wp[:, 0:8:4], in_=wdiff_r[:, 0:2])
    nc.vector.tensor_copy(out=wp[:, 1:8:4], in_=wdiff_r[:, 2:4])
    nc.vector.tensor_copy(out=wp[:, 2:8:4], in_=wdiff_c[:, 0:2])
    nc.vector.tensor_copy(out=wp[:, 3:8:4], in_=wdiff_c[:, 2:4])

    br_raw = const.tile([1, 2], F32, tag="br_raw")
    bc_raw = const.tile([1, 2], F32, tag="bc_raw")
    nc.sync.dma_start(out=br_raw[:], in_=b_row[:])
    nc.sync.dma_start(out=bc_raw[:], in_=b_col[:])
    db_rc = const.tile([1, 2], F32, tag="db_rc")
    nc.vector.tensor_tensor(
        out=db_rc[:, 0:1], in0=br_raw[:, 1:2], in1=br_raw[:, 0:1],
        op=mybir.AluOpType.subtract,
    )
    nc.vector.tensor_tensor(
        out=db_rc[:, 1:2], in0=bc_raw[:, 1:2], in1=bc_raw[:, 0:1],
        op=mybir.AluOpType.subtract,
    )
    # broadcast db/2 via a 0.5-valued ones row (tanh bias is db/2)
    half_row = const.tile([1, P], F32, tag="half_row")
    nc.vector.memset(half_row[:], 0.5)
    db_psum = psmall.tile([P, 2], F32, tag="db_psum")
    nc.tensor.matmul(
        db_psum[:], lhsT=half_row[:], rhs=db_rc[:], start=True, stop=True,
    )
    db128 = const.tile([P, 2], F32, tag="db128")
    nc.vector.tensor_copy(out=db128[:], in_=db_psum[:])
    return wp, db128


def _emit_offsets(nc, const, work, idx):
    """Load packed index lists and compute branch-table row offsets
    off(b, n) = (n%128)*128 + b*16 + n/128
    for e1=(pb,pi), e2=(pb,pj), e3=(nb,ni) per branch."""
    idx_sb = const.tile([GP, 10 * GK], I32, tag="idx_sb")
    nc.sync.dma_start(out=idx_sb[:], in_=idx[:])

    def off_tile(b_ap, n_ap, name):
        t_lo = work.tile([GP, GK], I32, tag=f"{name}_lo")
        t_hi = work.tile([GP, GK], I32, tag=f"{name}_hi")
        t_b = work.tile([GP, GK], I32, tag=f"{name}_b")
        out = const.tile([GP, GK], I32, tag=f"{name}_out")
        # (n & 127) * 128 == (n & 127) << 7; n / 128 == (n & ~127) >> 7
        # — keep each fused pair in one ALU class.
        nc.vector.tensor_scalar(
            out=t_lo[:], in0=n_ap, scalar1=127, scalar2=7,
            op0=mybir.AluOpType.bitwise_and,
            op1=mybir.AluOpType.logical_shift_left,
        )
        nc.vector.tensor_scalar(
            out=t_hi[:], in0=n_ap, scalar1=-128, scalar2=7,
            op0=mybir.AluOpType.bitwise_and,
            op1=mybir.AluOpType.logical_shift_right,
        )
        nc.vector.tensor_scalar(
            out=t_b[:], in0=b_ap, scalar1=16, scalar2=None,
            op0=mybir.AluOpType.mult,
        )
        nc.vector.tensor_tensor(
            out=t_lo[:], in0=t_lo[:], in1=t_hi[:], op=mybir.AluOpType.add,
        )
        nc.vector.tensor_tensor(
            out=out[:], in0=t_lo[:], in1=t_b[:], op=mybir.AluOpType.add,
        )
        return out

    def idx_list(branch, l):
        o = (branch * 5 + l) * GK
        return idx_sb[:, o:o + GK]

    offs = {}
    for br in (0, 1):
        pb, pi, pj, nb, ni = (idx_list(br, l) for l in range(5))
        offs[br, "e1"] = off_tile(pb, pi, f"b{br}e1")
        offs[br, "e2"] = off_tile(pb, pj, f"b{br}e2")
        offs[br, "e3"] = off_tile(nb, ni, f"b{br}e3")
    return idx_sb, offs


def _emit_gather_and_loss(nc, const, work, psmall, tables_sb, offs,
                          db128, partial):
    """GPSIMD ap_gather from per-partition-replicated bf16 branch
    tables + tanh/exp/ln row losses + reduction into the per-core
    partial.

    Each Q7 core (16 channels) gathers its own 1600 pair slots per
    stream; all 16 channels of a core return identical rows, so every
    pair is counted exactly 16x and the final scale divides by 16.
    Slots j >= N_REAL[ci] are pads (table row 0); their uniform
    contribution is computed from a known pad slot and subtracted."""
    # int16 index tiles (values < 16384 fit)
    stream_order = [(0, "e1"), (0, "e2"), (0, "e3"),
                    (1, "e1"), (1, "e2"), (1, "e3")]
    idx16 = const.tile([P, 600], I16, tag="idx16")
    for s, (br, e) in enumerate(stream_order):
        nc.vector.tensor_copy(
            out=idx16[:, s * GK:(s + 1) * GK], in_=offs[br, e][:],
        )

    NH = NI // 2  # 800 idxs per ap_gather call
    # acc: 12 sum(L) slots + 4 sum(y of pos rowset) slots, all f32
    acc = const.tile([P, 16], F32, tag="acc")
    bias_one = const.tile([P, 1], F32, tag="bias_one")
    nc.vector.memset(bias_one[:], 1.0)

    sidx = {(br, e): s for s, (br, e) in enumerate(stream_order)}
    pad_reads = {}
    i = 0
    for br in (0, 1):
        table_sb = tables_sb[br]
        for h in range(2):
            gath = {}
            for e in ("e1", "e2", "e3"):
                s = sidx[br, e]
                g_t = work.tile([P, NH, 2], BF16, tag="g", bufs=6)
                nc.gpsimd.ap_gather(
                    out_ap=g_t[:],
                    in_ap=table_sb[:].rearrange("p (e d) -> p e d", d=2),
                    idxs_ap=idx16[:, s * GK + h * 50: s * GK + h * 50 + 50],
                    channels=P, num_elems=TROWS_BR, d=2, num_idxs=NH,
                )
                gath[e] = g_t
            g1, g2, g3 = gath["e1"], gath["e2"], gath["e3"]
            for nm, in0, in1 in (
                ("pos", g1[:, :, 0], g2[:, :, 1]),
                ("negA", g3[:, :, 0], g1[:, :, 1]),
                ("negB", g3[:, :, 0], g2[:, :, 1]),
            ):
                d_t = work.tile([P, NH], BF16, tag="d", bufs=4)
                nc.vector.tensor_tensor(
                    out=d_t[:], in0=in0, in1=in1, op=mybir.AluOpType.add,
                )
                y_t = work.tile([P, NH], BF16, tag="y", bufs=3)
                kw = {}
                if nm == "pos":
                    kw["accum_out"] = acc[:, 12 + br * 2 + h:13 + br * 2 + h]
                nc.scalar.activation(
                    out=y_t[:], in_=d_t[:],
                    func=mybir.ActivationFunctionType.Tanh,
                    bias=db128[:, br:br + 1], scale=0.5, **kw,
                )
                e_t = work.tile([P, NH], BF16, tag="e", bufs=3)
                nc.scalar.activation(
                    out=e_t[:], in_=y_t[:],
                    func=mybir.ActivationFunctionType.Exp,
                )
                l_t = work.tile([P, NH], BF16, tag="l", bufs=3)
                nc.scalar.activation(
                    out=l_t[:], in_=e_t[:],
                    func=mybir.ActivationFunctionType.Ln,
                    bias=bias_one[:, 0:1], scale=1.0,
                    accum_out=acc[:, i:i + 1],
                )
                if nm == "pos" and h == 1:
                    pad_reads[br] = (l_t, y_t)
                i += 1

    # reduce: sum(L) - sum(y_pos)
    t_l = const.tile([P, 1], F32, tag="t_l")
    t_y = const.tile([P, 1], F32, tag="t_y")
    nc.vector.tensor_reduce(
        out=t_l[:], in_=acc[:, 0:12], axis=mybir.AxisListType.X,
        op=mybir.AluOpType.add,
    )
    nc.vector.tensor_reduce(
        out=t_y[:], in_=acc[:, 12:16], axis=mybir.AxisListType.X,
        op=mybir.AluOpType.add,
    )
    total = const.tile([P, 1], F32, tag="total")
    nc.vector.tensor_tensor(
        out=total[:], in0=t_l[:], in1=t_y[:], op=mybir.AluOpType.subtract,
    )
    # every channel counts its core's pairs once -> 16x redundancy
    scale_vec = const.tile([P, 1], F32, tag="scale_vec")
    nc.vector.memset(scale_vec[:], 1.0 / (3.0 * R * 16.0))
    out_psum = psmall.tile([1, 1], F32, tag="out_psum")
    nc.tensor.matmul(
        out_psum[:], lhsT=total[:], rhs=scale_vec[:], start=True, stop=True,
    )
    out_sb = const.tile([1, 1], F32, tag="out_sb")
    nc.vector.tensor_copy(out=out_sb[:], in_=out_psum[:])

    # pad correction: slot (p=0, f=48, h=1) is a pad (j=1568 >= N_REAL);
    # its out free position is 48*16+0 = 768.  All pads of branch br
    # share that branch's table row 0, so per pad pair and branch the
    # pollution is 3*L_br - y_br.  Total = 16ch * 300 pads; after the
    # 1/(3R*16) scale: 0.001 units.
    PADPOS = 768
    corr = const.tile([1, 4], F32, tag="corr")
    for br in (0, 1):
        l_t, y_t = pad_reads[br]
        nc.vector.tensor_copy(out=corr[0:1, br:br + 1],
                              in_=l_t[0:1, PADPOS:PADPOS + 1])
        nc.vector.tensor_copy(out=corr[0:1, 2 + br:3 + br],
                              in_=y_t[0:1, PADPOS:PADPOS + 1])
    cs = const.tile([1, 2], F32, tag="cs")
    nc.vector.tensor_tensor(out=cs[0:1, 0:1], in0=corr[0:1, 0:1],
                            in1=corr[0:1, 1:2], op=mybir.AluOpType.add)
    nc.vector.tensor_tensor(out=cs[0:1, 1:2], in0=corr[0:1, 2:3],
                            in1=corr[0:1, 3:4], op=mybir.AluOpType.add)
    cs2 = const.tile([1, 2], F32, tag="cs2")
    nc.vector.tensor_scalar(out=cs2[0:1, 0:1], in0=cs[0:1, 0:1],
                            scalar1=0.003, scalar2=None,
                            op0=mybir.AluOpType.mult)
    nc.vector.tensor_scalar(out=cs2[0:1, 1:2], in0=cs[0:1, 1:2],
                            scalar1=0.001, scalar2=None,
                            op0=mybir.AluOpType.mult)
    out2 = const.tile([1, 1], F32, tag="out2")
    nc.vector.tensor_tensor(out=out2[:], in0=out_sb[:], in1=cs2[0:1, 0:1],
                            op=mybir.AluOpType.subtract)
    nc.vector.tensor_tensor(out=out2[:], in0=out2[:], in1=cs2[0:1, 1:2],
                            op=mybir.AluOpType.add)
    nc.sync.dma_start(out=partial[:], in_=out2[0, :])


def _build_nc():
    """Replicated: every core loads all of all_features and builds both
    branch tables itself.  Branch table row = (n%128)*128 + b*16 + n/128,
    d = (lo, hi) bf16."""
    nc = bacc.Bacc()

    feats = nc.declare_dram_parameter("feats", [B, C, N], F32, isOutput=False)
    w_row = nc.declare_dram_parameter("w_row", [2 * C, 2], F32, isOutput=False)
    w_col = nc.declare_dram_parameter("w_col", [2 * C, 2], F32, isOutput=False)
    b_row = nc.declare_dram_parameter("b_row", [1, 2], F32, isOutput=False)
    b_col = nc.declare_dram_parameter("b_col", [1, 2], F32, isOutput=False)
    idx = nc.declare_dram_parameter("idx", [GP, 10 * GK], I32, isOutput=False)
    partial = nc.declare_dram_parameter("partial", [1], F32, isOutput=True)

    # flat bf16 branch tables in DRAM (64KB each)
    t_row_dram = nc.dram_tensor("t_row", [TELEMS_BR], BF16)
    t_col_dram = nc.dram_tensor("t_col", [TELEMS_BR], BF16)

    with tile.TileContext(nc) as tc:
        with (
            tc.tile_pool(name="const", bufs=1) as const,
            tc.tile_pool(name="fbpool", bufs=2) as fbpool,
            tc.tile_pool(name="work", bufs=2) as work,
            tc.tile_pool(name="psum", bufs=4, space="PSUM") as psum,
            tc.tile_pool(name="psmall", bufs=1, space="PSUM") as psmall,
        ):
            wp, db128 = _emit_weight_prep(nc, const, psmall, w_row, w_col,
                                          b_row, b_col)
            _, offs = _emit_offsets(nc, const, work, idx)

            # branch tables staged in SBUF as [q=128, b*16+blk, d]
            s_row = const.tile([P, B * 16 * 2], BF16, tag="s_row")
            s_col = const.tile([P, B * 16 * 2], BF16, tag="s_col")
            for b in range(B):
                fb = fbpool.tile([P, 2 * N], BF16, tag="fb")
                nc.gpsimd.dma_start(
                    out=fb[:].rearrange("p (kt n) -> p kt n", kt=2),
                    in_=feats[b].rearrange("(kt p) n -> p kt n", p=P),
                )
                pt = psum.tile([P, 64], F32, tag="pt")
                for blk in range(16):
                    nc.tensor.matmul(
                        pt[:, blk * 4:(blk + 1) * 4],
                        lhsT=fb[:, blk * P:(blk + 1) * P],
                        rhs=wp[:, 0:4], start=True, stop=False,
                    )
                    nc.tensor.matmul(
                        pt[:, blk * 4:(blk + 1) * 4],
                        lhsT=fb[:, N + blk * P:N + (blk + 1) * P],
                        rhs=wp[:, 4:8], start=False, stop=True,
                    )
                ptv = pt[:].rearrange("p (blk m) -> p blk m", m=4)
                nc.vector.tensor_copy(
                    out=s_row[:, b * 32:(b + 1) * 32].rearrange(
                        "p (blk d) -> p blk d", d=2),
                    in_=ptv[:, :, 0:2],
                )
                nc.vector.tensor_copy(
                    out=s_col[:, b * 32:(b + 1) * 32].rearrange(
                        "p (blk d) -> p blk d", d=2),
                    in_=ptv[:, :, 2:4],
                )
            w_r = nc.sync.dma_start(
                out=t_row_dram[:].rearrange("(q f) -> q f", q=P),
                in_=s_row[:],
            )
            w_c = nc.sync.dma_start(
                out=t_col_dram[:].rearrange("(q f) -> q f", q=P),
                in_=s_col[:],
            )

            # broadcast row table first, col strictly after, so row
            # gathers start as soon as possible
            table_row = const.tile([P, TELEMS_BR], BF16, tag="table_row")
            table_col = const.tile([P, TELEMS_BR], BF16, tag="table_col")
            bc_r = nc.sync.dma_start(
                out=table_row[:], in_=t_row_dram[:].partition_broadcast(P),
            )
            tile.add_dep_helper(bc_r.ins, w_r.ins, sync=True,
                                reason="broadcast after table write")
            bc_c = nc.sync.dma_start(
                out=table_col[:], in_=t_col_dram[:].partition_broadcast(P),
            )
            tile.add_dep_helper(bc_c.ins, w_c.ins, sync=True,
                                reason="broadcast after table write")
            tile.add_dep_helper(bc_c.ins, bc_r.ins, sync=True,
                                reason="serialize broadcasts: row first")

            _emit_gather_and_loss(nc, const, work, psmall,
                                  {0: table_row, 1: table_col}, offs,
                                  db128, partial)
    return nc


_NC_CACHE = {}


def _get_nc():
    if "nc" not in _NC_CACHE:
        nc = _build_nc()
        nc.finalize()  # Bacc: regalloc, event sems, ACT table loads
        _NC_CACHE["nc"] = nc
    return _NC_CACHE["nc"]


def _pack_core_inputs(inputs, core):
    lists = [
        inputs["row_pos_b"], inputs["row_pos_i"], inputs["row_pos_j"],
        inputs["row_neg_b"], inputs["row_neg_i"],
        inputs["col_pos_b"], inputs["col_pos_i"], inputs["col_pos_j"],
        inputs["col_neg_b"], inputs["col_neg_i"],
    ]
    base = core * PAIRS
    # device slot (p, F): Q7 core ci=p//16, lane u=p%16, chunk h=F//50,
    # f=F%50 -> core-local j = h*800 + f*16 + u; real pair for j <
    # N_REAL[ci], else pad (index 0, corrected on device).
    p = np.arange(P)[:, None]
    F = np.arange(GK)[None, :]
    ci, u = p // 16, p % 16
    h, f = F // 50, F % 50
    j = h * 800 + f * 16 + u
    nreal = np.array(N_REAL)
    cumb = np.concatenate([[0], np.cumsum(nreal)])[:-1]
    pair = cumb[ci] + j
    valid = j < nreal[ci]
    pair_c = np.clip(pair, 0, PAIRS - 1)
    arr = np.zeros((P, 10 * GK), np.int32)
    for l, lst in enumerate(lists):
        v = np.asarray(lst[base:base + PAIRS], np.int32)
        arr[:, l * GK:(l + 1) * GK] = np.where(valid, v[pair_c], 0)
    return {
        "feats": np.ascontiguousarray(
            np.asarray(inputs["all_features"], np.float32)),
        "w_row": np.ascontiguousarray(np.asarray(inputs["W_row"], np.float32)),
        "w_col": np.ascontiguousarray(np.asarray(inputs["W_col"], np.float32)),
        "b_row": np.ascontiguousarray(
            np.asarray(inputs["b_row"], np.float32).reshape(1, 2)),
        "b_col": np.ascontiguousarray(
            np.asarray(inputs["b_col"], np.float32).reshape(1, 2)),
        "idx": arr,
    }


def run(inputs, trace=False):
    nc = _get_nc()
    in_maps = [_pack_core_inputs(inputs, c) for c in range(NCORES)]
    res = run_bass_kernel_spmd(nc, in_maps, list(range(NCORES)), trace=trace)
    partials = np.array(
        [res.results[c]["partial"][0] for c in range(NCORES)], np.float32
    )
    out = np.array([partials.sum()], np.float32)
    return out, res


def kernel(**inputs):
    out, _ = run(inputs, trace=False)
    return out


# revision 6
# speedup vs baseline: 4.8485x; 4.8485x over previous
"""Trainium2 Bass kernel for the pairwise-classifier loss.

Math: per branch, logits = x @ W + b with only 2 classes, so everything
reduces to the logit difference d = x . (W[:,1]-W[:,0]) + (b[1]-b[0]).
With x a concat of two gathered feature vectors, d splits into a sum of
two per-position projections:
    lo[b,n] = sum_c feats[b,c,n] * w[c]      (w = first 256 rows of dW)
    hi[b,n] = sum_c feats[b,c,n] * w[256+c]  (last 256 rows)
Per pair r: d_pos = lo[pb,pi] + hi[pb,pj] + db
            d_negA = lo[nb,ni] + hi[pb,pi] + db
            d_negB = lo[nb,ni] + hi[pb,pj] + db
and the double-softmax CE row loss, with y = tanh((d+db)/2)
(= 2*sigmoid(d+db)-1), E = exp(y), L = ln(E+1):
    loss_neg = L           (softplus(y))
    loss_pos = L - y       (softplus(-y))
The permutation inputs are irrelevant: the loss is a mean over rows.

Device plan (8 cores, pairs data-parallel 12500/core, replicated
feature load -> no collectives):
  1. Each core streams all 16MB of features (f32->bf16 cast in DMA),
     8 per-b chunks multi-buffered.
  2. PE per b: feats_block[128c,128n]^T @ Wp[128c,4] -> PSUM [128n,4m],
     m = (row_lo,row_hi,col_lo,col_hi); 2 k-tiles accumulated.  DVE
     de-interleaves row/col into two bf16 branch tables in SBUF, DMAed
     to DRAM as [16384, 2] tables: row index (n%128)*128 + b*16 + n/128,
     d=2 (lo,hi).
  3. Flat table row indices via DVE int ops; 6 SWDGE indirect-DMA
     gathers (one per endpoint stream) of 4-byte (lo,hi) bf16 rows
     straight from the DRAM tables into [128, 98, 2] SBUF tiles -- one
     row per pair slot, no lane redundancy and no table broadcast.
  4. DVE adds form d tiles; ACT tanh/exp/ln with per-partition
     accum_out.  Pad slots (44 per stream) replicate table row 0; their
     uniform contribution is computed from a known pad slot and
     subtracted.  PE dot with the 1/(3R) scale vector -> per-core
     partial; host sums the 8 partials.
"""

import os

import numpy as np

import concourse.bass as bass
import concourse.bacc as bacc
import concourse.mybir as mybir
import concourse.tile as tile
from concourse.bass_utils import run_bass_kernel_spmd

F32 = mybir.dt.float32
BF16 = mybir.dt.bfloat16
I32 = mybir.dt.int32

B, C, N = 8, 256, 2048
R = 100000
NCORES = 8
PAIRS = R // NCORES          # 12500 pairs per core per branch
P = 128
GK = 98                      # idx free width; 128*98 = 12544 >= 12500
NPAD = P * GK - PAIRS        # 44 pad slots per stream
TROWS_BR = P * B * (N // P)  # 16384 (q, b, blk) rows per branch table


def _emit_weight_prep(nc, const, psmall, w_row, w_col, b_row, b_col):
    """Wp [128, 2kt*4m] bf16 projection weights and db128 [128, 2] f32
    holding (b[1]-b[0])/2 per branch broadcast to all partitions."""
    wr_raw = const.tile([P, 8], F32, tag="wr_raw")
    wc_raw = const.tile([P, 8], F32, tag="wc_raw")
    nc.sync.dma_start(
        out=wr_raw[:].rearrange("p (s t) -> p s t", s=4),
        in_=w_row[:].rearrange("(s p) t -> p s t", p=P),
    )
    nc.sync.dma_start(
        out=wc_raw[:].rearrange("p (s t) -> p s t", s=4),
        in_=w_col[:].rearrange("(s p) t -> p s t", p=P),
    )
    wdiff_r = const.tile([P, 4], F32, tag="wdiff_r")
    wdiff_c = const.tile([P, 4], F32, tag="wdiff_c")
    nc.vector.tensor_tensor(
        out=wdiff_r[:], in0=wr_raw[:, 1::2], in1=wr_raw[:, 0::2],
        op=mybir.AluOpType.subtract,
    )
    nc.vector.tensor_tensor(
        out=wdiff_c[:], in0=wc_raw[:, 1::2], in1=wc_raw[:, 0::2],
        op=mybir.AluOpType.subtract,
    )
    # Wp[:, kt*4 + m]: m = (row_lo, row_hi, col_lo, col_hi)
    wp = const.tile([P, 8], BF16, tag="wp")
    nc.vector.tensor_copy(out=wp[:, 0:8:4], in_=wdiff_r[:, 0:2])
    nc.vector.tensor_copy(out=wp[:, 1:8:4], in_=wdiff_r[:, 2:4])
    nc.vector.tensor_copy(out=wp[:, 2:8:4], in_=wdiff_c[:, 0:2])
    nc.vector.tensor_copy(out=wp[:, 3:8:4], in_=wdiff_c[:, 2:4])

    br_raw = const.tile([1, 2], F32, tag="br_raw")
    bc_raw = const.tile([1, 2], F32, tag="bc_raw")
    nc.sync.dma_start(out=br_raw[:], in_=b_row[:])
    nc.sync.dma_start(out=bc_raw[:], in_=b_col[:])
    db_rc = const.tile([1, 2], F32, tag="db_rc")
    nc.vector.tensor_tensor(
        out=db_rc[:, 0:1], in0=br_raw[:, 1:2], in1=br_raw[:, 0:1],
        op=mybir.AluOpType.subtract,
    )
    nc.vector.tensor_tensor(
        out=db_rc[:, 1:2], in0=bc_raw[:, 1:2], in1=bc_raw[:, 0:1],
        op=mybir.AluOpType.subtract,
    )
    # broadcast db/2 via a 0.5-valued ones row (tanh bias is db/2)
    half_row = const.tile([1, P], F32, tag="half_row")
    nc.vector.memset(half_row[:], 0.5)
    db_psum = psmall.tile([P, 2], F32, tag="db_psum")
    nc.tensor.matmul(
        db_psum[:], lhsT=half_row[:], rhs=db_rc[:], start=True, stop=True,
    )
    db128 = const.tile([P, 2], F32, tag="db128")
    nc.vector.tensor_copy(out=db128[:], in_=db_psum[:])
    return wp, db128


def _emit_offsets(nc, const, work, idx):
    """Load packed index lists and compute branch-table row indices
    off(b, n) = (n%128)*128 + b*16 + n/128
    for e1=(pb,pi), e2=(pb,pj), e3=(nb,ni) per branch."""
    idx_sb = const.tile([P, 10 * GK], I32, tag="idx_sb")
    nc.sync.dma_start(out=idx_sb[:], in_=idx[:])

    def off_tile(b_ap, n_ap, name):
        t_lo = work.tile([P, GK], I32, tag=f"{name}_lo")
        t_hi = work.tile([P, GK], I32, tag=f"{name}_hi")
        t_b = work.tile([P, GK], I32, tag=f"{name}_b")
        out = const.tile([P, GK], I32, tag=f"{name}_out")
        # (n & 127) * 128 == (n & 127) << 7; n / 128 == (n & ~127) >> 7
        # — keep each fused pair in one ALU class.
        nc.vector.tensor_scalar(
            out=t_lo[:], in0=n_ap, scalar1=127, scalar2=7,
            op0=mybir.AluOpType.bitwise_and,
            op1=mybir.AluOpType.logical_shift_left,
        )
        nc.vector.tensor_scalar(
            out=t_hi[:], in0=n_ap, scalar1=-128, scalar2=7,
            op0=mybir.AluOpType.bitwise_and,
            op1=mybir.AluOpType.logical_shift_right,
        )
        nc.vector.tensor_scalar(
            out=t_b[:], in0=b_ap, scalar1=16, scalar2=None,
            op0=mybir.AluOpType.mult,
        )
        nc.vector.tensor_tensor(
            out=t_lo[:], in0=t_lo[:], in1=t_hi[:], op=mybir.AluOpType.add,
        )
        nc.vector.tensor_tensor(
            out=out[:], in0=t_lo[:], in1=t_b[:], op=mybir.AluOpType.add,
        )
        return out

    def idx_list(branch, l):
        o = (branch * 5 + l) * GK
        return idx_sb[:, o:o + GK]

    offs = {}
    for br in (0, 1):
        pb, pi, pj, nb, ni = (idx_list(br, l) for l in range(5))
        offs[br, "e1"] = off_tile(pb, pi, f"b{br}e1")
        offs[br, "e2"] = off_tile(pb, pj, f"b{br}e2")
        offs[br, "e3"] = off_tile(nb, ni, f"b{br}e3")
    return idx_sb, offs


def _emit_gather_and_loss(nc, const, work, psmall, tables_dram, offs,
                          db128, partial, table_writes):
    """SWDGE indirect-DMA gathers from the DRAM branch tables +
    tanh/exp/ln row losses + reduction into the per-core partial.

    Pair slot (p, f) holds pair j = p*98 + f; slots j >= 12500 are pads
    (table row 0); their uniform contribution is computed from the last
    slot (p=127, f=97) and subtracted."""
    # acc: 6 sum(L) slots + 2 sum(y of pos rowset) slots, all f32
    acc = const.tile([P, 8], F32, tag="acc")
    bias_one = const.tile([P, 1], F32, tag="bias_one")
    nc.vector.memset(bias_one[:], 1.0)

    pad_reads = {}
    i = 0
    for br in (0, 1):
        t_dram = tables_dram[br]
        gath = {}
        for e in ("e1", "e2", "e3"):
            g_t = work.tile([P, GK, 2], BF16, tag=f"g{br}{e}")
            ind = nc.gpsimd.indirect_dma_start(
                out=g_t[:],
                out_offset=None,
                in_=t_dram[:],
                in_offset=bass.IndirectOffsetOnAxis(
                    ap=offs[br, e][:], axis=0,
                ),
            )
            tile.add_dep_helper(ind.ins, table_writes[br].ins, sync=True,
                                reason="gather after table write")
            gath[e] = g_t
        g1, g2, g3 = gath["e1"], gath["e2"], gath["e3"]
        for nm, in0, in1 in (
            ("pos", g1[:, :, 0], g2[:, :, 1]),
            ("negA", g3[:, :, 0], g1[:, :, 1]),
            ("negB", g3[:, :, 0], g2[:, :, 1]),
        ):
            d_t = work.tile([P, GK], BF16, tag="d", bufs=3)
            nc.vector.tensor_tensor(
                out=d_t[:], in0=in0, in1=in1, op=mybir.AluOpType.add,
            )
            y_t = work.tile([P, GK], BF16, tag="y", bufs=3)
            kw = {}
            if nm == "pos":
                kw["accum_out"] = acc[:, 6 + br:7 + br]
            nc.scalar.activation(
                out=y_t[:], in_=d_t[:],
                func=mybir.ActivationFunctionType.Tanh,
                bias=db128[:, br:br + 1], scale=0.5, **kw,
            )
            e_t = work.tile([P, GK], BF16, tag="e", bufs=3)
            nc.scalar.activation(
                out=e_t[:], in_=y_t[:],
                func=mybir.ActivationFunctionType.Exp,
            )
            l_t = work.tile([P, GK], BF16, tag="l", bufs=3)
            nc.scalar.activation(
                out=l_t[:], in_=e_t[:],
                func=mybir.ActivationFunctionType.Ln,
                bias=bias_one[:, 0:1], scale=1.0,
                accum_out=acc[:, i:i + 1],
            )
            if nm == "pos":
                pad_reads[br] = (l_t, y_t)
            i += 1

    # reduce: sum(L) - sum(y_pos)
    t_l = const.tile([P, 1], F32, tag="t_l")
    t_y = const.tile([P, 1], F32, tag="t_y")
    nc.vector.tensor_reduce(
        out=t_l[:], in_=acc[:, 0:6], axis=mybir.AxisListType.X,
        op=mybir.AluOpType.add,
    )
    nc.vector.tensor_reduce(
        out=t_y[:], in_=acc[:, 6:8], axis=mybir.AxisListType.X,
        op=mybir.AluOpType.add,
    )
    total = const.tile([P, 1], F32, tag="total")
    nc.vector.tensor_tensor(
        out=total[:], in0=t_l[:], in1=t_y[:], op=mybir.AluOpType.subtract,
    )
    scale_vec = const.tile([P, 1], F32, tag="scale_vec")
    nc.vector.memset(scale_vec[:], 1.0 / (3.0 * R))
    out_psum = psmall.tile([1, 1], F32, tag="out_psum")
    nc.tensor.matmul(
        out_psum[:], lhsT=total[:], rhs=scale_vec[:], start=True, stop=True,
    )
    out_sb = const.tile([1, 1], F32, tag="out_sb")
    nc.vector.tensor_copy(out=out_sb[:], in_=out_psum[:])

    # pad correction: slots with j = (127-p)*98 + f >= 12500 are pads;
    # slot (p=0, f=97) is one.  All pads of branch br share that
    # branch's table row 0, so per pad pair the pollution is
    # 3*L_br - y_br.  44 pads per stream; scale 1/(3R).
    corr = const.tile([1, 4], F32, tag="corr")
    for br in (0, 1):
        l_t, y_t = pad_reads[br]
        nc.vector.tensor_copy(out=corr[0:1, br:br + 1],
                              in_=l_t[0:1, GK - 1:GK])
        nc.vector.tensor_copy(out=corr[0:1, 2 + br:3 + br],
                              in_=y_t[0:1, GK - 1:GK])
    cs = const.tile([1, 2], F32, tag="cs")
    nc.vector.tensor_tensor(out=cs[0:1, 0:1], in0=corr[0:1, 0:1],
                            in1=corr[0:1, 1:2], op=mybir.AluOpType.add)
    nc.vector.tensor_tensor(out=cs[0:1, 1:2], in0=corr[0:1, 2:3],
                            in1=corr[0:1, 3:4], op=mybir.AluOpType.add)
    cs2 = const.tile([1, 2], F32, tag="cs2")
    nc.vector.tensor_scalar(out=cs2[0:1, 0:1], in0=cs[0:1, 0:1],
                            scalar1=3.0 * NPAD / (3.0 * R), scalar2=None,
                            op0=mybir.AluOpType.mult)
    nc.vector.tensor_scalar(out=cs2[0:1, 1:2], in0=cs[0:1, 1:2],
                            scalar1=1.0 * NPAD / (3.0 * R), scalar2=None,
                            op0=mybir.AluOpType.mult)
    out2 = const.tile([1, 1], F32, tag="out2")
    nc.vector.tensor_tensor(out=out2[:], in0=out_sb[:], in1=cs2[0:1, 0:1],
                            op=mybir.AluOpType.subtract)
    nc.vector.tensor_tensor(out=out2[:], in0=out2[:], in1=cs2[0:1, 1:2],
                            op=mybir.AluOpType.add)
    nc.sync.dma_start(out=partial[:], in_=out2[0, :])


def _build_nc():
    """Replicated: every core loads all of all_features and builds both
    branch tables itself.  Branch table row = (n%128)*128 + b*16 + n/128,
    d = (lo, hi) bf16."""
    nc = bacc.Bacc()

    feats = nc.declare_dram_parameter("feats", [B, C, N], F32, isOutput=False)
    w_row = nc.declare_dram_parameter("w_row", [2 * C, 2], F32, isOutput=False)
    w_col = nc.declare_dram_parameter("w_col", [2 * C, 2], F32, isOutput=False)
    b_row = nc.declare_dram_parameter("b_row", [1, 2], F32, isOutput=False)
    b_col = nc.declare_dram_parameter("b_col", [1, 2], F32, isOutput=False)
    idx = nc.declare_dram_parameter("idx", [P, 10 * GK], I32, isOutput=False)
    partial = nc.declare_dram_parameter("partial", [1], F32, isOutput=True)

    # bf16 branch tables in DRAM (64KB each)
    t_row_dram = nc.dram_tensor("t_row", [TROWS_BR, 2], BF16)
    t_col_dram = nc.dram_tensor("t_col", [TROWS_BR, 2], BF16)

    with tile.TileContext(nc) as tc:
        with (
            tc.tile_pool(name="const", bufs=1) as const,
            tc.tile_pool(name="fbpool", bufs=3) as fbpool,
            tc.tile_pool(name="work", bufs=2) as work,
            tc.tile_pool(name="psum", bufs=4, space="PSUM") as psum,
            tc.tile_pool(name="psmall", bufs=1, space="PSUM") as psmall,
        ):
            wp, db128 = _emit_weight_prep(nc, const, psmall, w_row, w_col,
                                          b_row, b_col)
            _, offs = _emit_offsets(nc, const, work, idx)

            # branch tables staged in SBUF as [q=128, b*16+blk, d]
            s_row = const.tile([P, B * 16 * 2], BF16, tag="s_row")
            s_col = const.tile([P, B * 16 * 2], BF16, tag="s_col")
            for b in range(B):
                fb = fbpool.tile([P, 2 * N], BF16, tag="fb")
                nc.gpsimd.dma_start(
                    out=fb[:].rearrange("p (kt n) -> p kt n", kt=2),
                    in_=feats[b].rearrange("(kt p) n -> p kt n", p=P),
                )
                pt = psum.tile([P, 64], F32, tag="pt")
                for blk in range(16):
                    nc.tensor.matmul(
                        pt[:, blk * 4:(blk + 1) * 4],
                        lhsT=fb[:, blk * P:(blk + 1) * P],
                        rhs=wp[:, 0:4], start=True, stop=False,
                    )
                    nc.tensor.matmul(
                        pt[:, blk * 4:(blk + 1) * 4],
                        lhsT=fb[:, N + blk * P:N + (blk + 1) * P],
                        rhs=wp[:, 4:8], start=False, stop=True,
                    )
                ptv = pt[:].rearrange("p (blk m) -> p blk m", m=4)
                nc.vector.tensor_copy(
                    out=s_row[:, b * 32:(b + 1) * 32].rearrange(
                        "p (blk d) -> p blk d", d=2),
                    in_=ptv[:, :, 0:2],
                )
                nc.vector.tensor_copy(
                    out=s_col[:, b * 32:(b + 1) * 32].rearrange(
                        "p (blk d) -> p blk d", d=2),
                    in_=ptv[:, :, 2:4],
                )
            w_r = nc.sync.dma_start(
                out=t_row_dram[:].rearrange("(q f) t -> q (f t)", q=P),
                in_=s_row[:],
            )
            w_c = nc.sync.dma_start(
                out=t_col_dram[:].rearrange("(q f) t -> q (f t)", q=P),
                in_=s_col[:],
            )

            _emit_gather_and_loss(nc, const, work, psmall,
                                  {0: t_row_dram, 1: t_col_dram}, offs,
                                  db128, partial, {0: w_r, 1: w_c})
    return nc


_NC_CACHE = {}


def _get_nc():
    if "nc" not in _NC_CACHE:
        nc = _build_nc()
        nc.finalize()  # Bacc: regalloc, event sems, ACT table loads
        _NC_CACHE["nc"] = nc
    return _NC_CACHE["nc"]


def _pack_core_inputs(inputs, core):
    lists = [
        inputs["row_pos_b"], inputs["row_pos_i"], inputs["row_pos_j"],
        inputs["row_neg_b"], inputs["row_neg_i"],
        inputs["col_pos_b"], inputs["col_pos_i"], inputs["col_pos_j"],
        inputs["col_neg_b"], inputs["col_neg_i"],
    ]
    base = core * PAIRS
    # device slot (p, f): pair j = (127-p)*98 + f; real for j < 12500,
    # else pad (index 0, corrected on device).  The reversed partition
    # order puts the pad slots on partition 0, where the engines can
    # read the known pad value (p=0, f=97) for the correction.
    p = np.arange(P)[:, None]
    f = np.arange(GK)[None, :]
    j = (P - 1 - p) * GK + f
    valid = j < PAIRS
    pair_c = np.clip(j, 0, PAIRS - 1)
    arr = np.zeros((P, 10 * GK), np.int32)
    for l, lst in enumerate(lists):
        v = np.asarray(lst[base:base + PAIRS], np.int32)
        arr[:, l * GK:(l + 1) * GK] = np.where(valid, v[pair_c], 0)
    return {
        "feats": np.ascontiguousarray(
            np.asarray(inputs["all_features"], np.float32)),
        "w_row": np.ascontiguousarray(np.asarray(inputs["W_row"], np.float32)),
        "w_col": np.ascontiguousarray(np.asarray(inputs["W_col"], np.float32)),
        "b_row": np.ascontiguousarray(
            np.asarray(inputs["b_row"], np.float32).reshape(1, 2)),
        "b_col": np.ascontiguousarray(
            np.asarray(inputs["b_col"], np.float32).reshape(1, 2)),
        "idx": arr,
    }


def run(inputs, trace=False):
    nc = _get_nc()
    in_maps = [_pack_core_inputs(inputs, c) for c in range(NCORES)]
    res = run_bass_kernel_spmd(nc, in_maps, list(range(NCORES)), trace=trace)
    partials = np.array(
        [res.results[c]["partial"][0] for c in range(NCORES)], np.float32
    )
    out = np.array([partials.sum()], np.float32)
    return out, res


def kernel(**inputs):
    out, _ = run(inputs, trace=False)
    return out
